# revision 24
# baseline (speedup 1.0000x reference)
"""LocalVarianceMap Trainium2 kernel (v2: halo-free tiling, warm PE).

reference:
  lum  = mean over channel of x            (B,1,H,W)
  mean = 7x7 'same' box mean of lum ; sqm = same of lum^2
  out  = sqm - mean^2

Full input x: (16, 3, 1024, 1024) fp32. Data-parallel over batch:
8 NeuronCores x 2 images each.

Per-core: 16 non-overlapping 128-row tiles (2 images x 8). Per tile:
  sync : load x0 -> lum (padded ring), x1 -> X1, x2 -> X2; store V -> y
  gpsimd: lum += X1 ; lum += X2            (channel sum, 2 TT adds)
  ACT  : sq = Square(lum) ; m2 = Square(S1/147) ; V = Copy(S2/441)
  DVE  : h1 = width-7 box scan of lum ; h2 = same of sq  (full 1027-col scans)
  PE   : S1 = Wmain^T h1(t) + Wlo^T h1(t-1) + Whi^T h1(t+1)   (f32r, per 512 half)
         S2 = same on h2, then += (-441 I)^T m2               (fused mean^2 sub)

The vertical 7-sum halo (3 rows each side) comes from the NEIGHBOR tiles'
h tensors via the Wlo/Whi corner-band matmuls accumulating into the same
PSUM bank, so no input rows are ever re-read or re-scanned. Warm PE runs
f32r matmuls at ~1 cyc/col, making the extra matmuls cheap.
"""

import sys

if "/opt/trn_rl_repo" not in sys.path:
    sys.path.insert(0, "/opt/trn_rl_repo")

import numpy as np
from contextlib import ExitStack

import concourse.bass as bass
import concourse.bacc as bacc
import concourse.tile as tile
from concourse import mybir

H = 1024
W = 1024
C = 3
PER_CORE_B = 2
N_CORES = 8
K7 = 7
PADL, PADR = 7, 3
LW = PADL + W + PADR      # padded lum/sq width (1034)
SCAN_N = W + 3            # h[:, j+3] = centered 7-sum at col j
NTILE = 16                # 2 images x 8 tiles of 128 rows
NRING = 8                 # lum ring depth
NSRING = 4                # sq ring depth
NHRING = 7                # h1/h2 ring depth (read by t-1, t, t+1 matmuls)


def band_weights() -> np.ndarray:
    """Blocks: Wmain | Wlo | Whi | -441*I, each [128,128]."""
    wb = np.zeros((128, 4 * 128), np.float32)
    for m in range(128):
        for k in range(max(m - 3, 0), min(m + 4, 128)):
            wb[k, m] = 1.0
    for m in range(3):  # Wlo: taps from prev tile rows 125+m..127
        for k in range(125 + m, 128):
            wb[k, 128 + m] = 1.0
    for m in range(125, 128):  # Whi: taps from next tile rows 0..m-125
        for k in range(0, m - 124):
            wb[k, 256 + m] = 1.0
    for m in range(128):
        wb[m, 384 + m] = -441.0
    return wb


def build_nc(finalize: bool = True) -> bass.Bass:
    nc = bacc.Bacc("TRN2", target_bir_lowering=False)
    f32 = mybir.dt.float32
    f32r = mybir.dt.float32r

    x = nc.dram_tensor("x", [PER_CORE_B, C, H, W], f32, kind="ExternalInput")
    wbt = nc.dram_tensor("wb", [128, 4 * 128], f32r, kind="ExternalInput")
    y = nc.dram_tensor("y", [PER_CORE_B, 1, H, W], f32, kind="ExternalOutput")

    inv147 = float(np.float32(1.0) / np.float32(147.0))
    inv441 = float(np.float32(1.0) / np.float32(441.0))

    with tile.TileContext(nc) as tc, ExitStack() as ctx:
        cpool = ctx.enter_context(tc.tile_pool(name="const", bufs=1))
        xpool = ctx.enter_context(tc.tile_pool(name="xch", bufs=8))
        mpool = ctx.enter_context(tc.tile_pool(name="m2", bufs=2))
        vpool = ctx.enter_context(tc.tile_pool(name="vout", bufs=2))
        p1pool = ctx.enter_context(tc.tile_pool(name="ps1", bufs=2, space="PSUM"))
        p2pool = ctx.enter_context(tc.tile_pool(name="ps2", bufs=2, space="PSUM"))

        WB = cpool.tile([128, 4 * 128], f32r)
        nc.sync.dma_start(out=WB[:], in_=wbt[:, :])

        # Persistent padded lum/sq rings; zero pads memset once.
        lum_ring = [
            cpool.tile([128, LW], f32, tag=f"lumr{i}", name=f"lumr{i}")
            for i in range(NRING)
        ]
        sq_ring = [
            cpool.tile([128, LW], f32, tag=f"sqr{i}", name=f"sqr{i}")
            for i in range(NSRING)
        ]
        h1_ring = [
            cpool.tile([128, SCAN_N], f32r, tag=f"h1r{i}", name=f"h1r{i}")
            for i in range(NHRING)
        ]
        h2_ring = [
            cpool.tile([128, SCAN_N], f32r, tag=f"h2r{i}", name=f"h2r{i}")
            for i in range(NHRING)
        ]
        for t_ in lum_ring + sq_ring:
            nc.gpsimd.memset(t_[:, 0:PADL], 0.0)
            nc.gpsimd.memset(t_[:, PADL + W : LW], 0.0)

        X1s, X2s, S1s, S2s, M2s, Vs = {}, {}, {}, {}, {}, {}

        def img_row(t):
            return t // 8, 128 * (t % 8)

        RAMP_T = 2   # first tiles skip the high-latency SWDGE accumulate
        SPLIT = 256  # x2-add columns done on DVE; rest on (lockout-stretched) gpsimd

        def st0_load(t):
            b, r0 = img_row(t)
            lum = lum_ring[t % NRING]
            nc.sync.dma_start(
                out=lum[:, PADL : PADL + W], in_=x[b, 0, r0 : r0 + 128, :]
            )
            X2 = xpool.tile([128, W], f32, tag="X2", name=f"X2_{t}")
            nc.sync.dma_start(out=X2[:, :], in_=x[b, 2, r0 : r0 + 128, :])
            X2s[t] = X2
            if t < RAMP_T:
                X1 = xpool.tile([128, W], f32, tag="X1", name=f"X1_{t}")
                nc.sync.dma_start(out=X1[:, :], in_=x[b, 1, r0 : r0 + 128, :])
                X1s[t] = X1

        def st1a_add(t):
            # x1 folded in during the DMA itself (SWDGE accumulate)
            b, r0 = img_row(t)
            lum = lum_ring[t % NRING]
            if t < RAMP_T:
                nc.gpsimd.tensor_add(
                    lum[:, PADL : PADL + W],
                    lum[:, PADL : PADL + W],
                    X1s.pop(t)[:, :],
                )
            else:
                nc.gpsimd.dma_start(
                    out=lum[:, PADL : PADL + W],
                    in_=x[b, 1, r0 : r0 + 128, :],
                    accum_op=mybir.AluOpType.add,
                )

        def st1b_add(t):
            # x2 add split: small slice on DVE, big slice on gpsimd, balancing
            # reduced DVE work against gpsimd's lockout-stretched rate.
            lum = lum_ring[t % NRING]
            X2 = X2s.pop(t)
            nc.vector.tensor_add(
                lum[:, PADL : PADL + SPLIT],
                lum[:, PADL : PADL + SPLIT],
                X2[:, 0:SPLIT],
            )
            nc.gpsimd.tensor_add(
                lum[:, PADL + SPLIT : PADL + W],
                lum[:, PADL + SPLIT : PADL + W],
                X2[:, SPLIT:W],
            )

        def st2_sq(t):
            lum = lum_ring[t % NRING]
            sq = sq_ring[t % NSRING]
            nc.scalar.activation(
                sq[:, PADL : PADL + W],
                lum[:, PADL : PADL + W],
                mybir.ActivationFunctionType.Square,
            )

        def st3_scan(t):
            lum = lum_ring[t % NRING]
            sq = sq_ring[t % NSRING]
            h1 = h1_ring[t % NHRING]
            h2 = h2_ring[t % NHRING]
            # h[:, j] = sum src[j-6..j]; matmul reads h[:, n+3] = centered sum
            for h, src in ((h1, lum), (h2, sq)):
                nc.vector.tensor_tensor_scan(
                    out=h[:, 0:SCAN_N],
                    data0=src[:, PADL : PADL + SCAN_N],
                    data1=src[:, 0:SCAN_N],
                    initial=0.0,
                    op0=mybir.AluOpType.add,
                    op1=mybir.AluOpType.subtract,
                )

        def _banded(S, hring, t, extra=None):
            """S[:, half] = Wmain^T h(t) + Wlo^T h(t-1) + Whi^T h(t+1) [+ extra]."""
            ti = t % 8
            for hf in range(2):
                lo, hi = 3 + 512 * hf, 3 + 512 * hf + 512
                parts = [(0, hring[t % NHRING])]
                if ti > 0:
                    parts.append((1, hring[(t - 1) % NHRING]))
                if ti < 7:
                    parts.append((2, hring[(t + 1) % NHRING]))
                n = len(parts) + (1 if extra is not None else 0)
                for j, (wsel, hsrc) in enumerate(parts):
                    nc.tensor.matmul(
                        S[0:128, 512 * hf : 512 * hf + 512],
                        WB[0:128, 128 * wsel : 128 * wsel + 128],
                        hsrc[0:128, lo:hi],
                        start=(j == 0),
                        stop=(j == n - 1),
                    )
                if extra is not None:
                    nc.tensor.matmul(
                        S[0:128, 512 * hf : 512 * hf + 512],
                        WB[0:128, 384:512],
                        extra[0:128, 512 * hf : 512 * hf + 512],
                        start=False,
                        stop=True,
                    )

        def st4_mm1(t):
            S1 = p1pool.tile([128, W], f32, tag="S1", name=f"S1_{t}")
            _banded(S1, h1_ring, t)
            S1s[t] = S1

        def st5_m2(t):
            m2 = mpool.tile([128, W], f32r, tag="m2", name=f"m2_{t}")
            nc.scalar.activation(
                m2[:, :],
                S1s.pop(t)[:, :],
                mybir.ActivationFunctionType.Square,
                scale=inv147,
            )
            M2s[t] = m2

        def st6_mm2(t):
            S2 = p2pool.tile([128, W], f32, tag="S2", name=f"S2_{t}")
            _banded(S2, h2_ring, t, extra=M2s.pop(t))
            S2s[t] = S2

        def st7_v(t):
            V = vpool.tile([128, W], f32, tag="V", name=f"V_{t}")
            nc.scalar.activation(
                V[:, :],
                S2s.pop(t)[:, :],
                mybir.ActivationFunctionType.Copy,
                scale=inv441,
            )
            Vs[t] = V

        def st8_out(t):
            # ACT-ring HWDGE right after ACT's own V copy: the trigger's
            # data dependency is already satisfied in-order, so it never
            # stalls the scalar queue (unlike a cross-engine wait would).
            b, r0 = img_row(t)
            nc.scalar.dma_start(
                out=y[b, 0, r0 : r0 + 128, :], in_=Vs.pop(t)[0:128, :]
            )

        # Software-pipelined emission with ~2 iterations of slack per stage.
        # Stage s of tile i-s runs in iteration i. On DVE the early-stage
        # x1-add is emitted before the older tile's scans (no head-blocking).
        for i in range(NTILE + 9):
            if i < NTILE:
                st0_load(i)
            if 1 <= i < NTILE + 1:
                st1a_add(i - 1)
            if 2 <= i < NTILE + 2:
                st1b_add(i - 2)
            if 3 <= i < NTILE + 3:
                st2_sq(i - 3)
            if 4 <= i < NTILE + 4:
                st3_scan(i - 4)
            if 6 <= i < NTILE + 6:
                st4_mm1(i - 6)
            if 7 <= i < NTILE + 7:
                st5_m2(i - 7)
            if 8 <= i < NTILE + 8:
                st6_mm2(i - 8)
            if 9 <= i < NTILE + 9:
                st7_v(i - 9)
                st8_out(i - 9)

    if finalize:
        nc.finalize()
    return nc


def kernel(x, kernel_size):
    assert int(kernel_size) == K7
    x = np.ascontiguousarray(np.asarray(x, dtype=np.float32))
    B = x.shape[0]
    assert x.shape == (B, C, H, W) and B == PER_CORE_B * N_CORES

    from concourse.bass_utils import run_bass_kernel_spmd

    nc = build_nc()
    wb = band_weights()
    in_maps = [
        {"x": x[i * PER_CORE_B : (i + 1) * PER_CORE_B], "wb": wb}
        for i in range(N_CORES)
    ]
    res = run_bass_kernel_spmd(nc, in_maps, list(range(N_CORES)))
    y = np.concatenate([res.results[i]["y"] for i in range(N_CORES)], axis=0)
    return y


# revision 28
# speedup vs baseline: 1.0650x; 1.0650x over previous
"""LocalVarianceMap Trainium2 kernel (v2: halo-free tiling, warm PE).

reference:
  lum  = mean over channel of x            (B,1,H,W)
  mean = 7x7 'same' box mean of lum ; sqm = same of lum^2
  out  = sqm - mean^2

Full input x: (16, 3, 1024, 1024) fp32. Data-parallel over batch:
8 NeuronCores x 2 images each.

Per-core: 16 non-overlapping 128-row tiles (2 images x 8). Per tile:
  sync : load x0 -> lum (padded ring), x1 -> X1, x2 -> X2; store V -> y
  gpsimd: lum += X1 ; lum += X2            (channel sum, 2 TT adds)
  ACT  : sq = Square(lum) ; m2 = Square(S1/147) ; V = Copy(S2/441)
  DVE  : h1 = width-7 box scan of lum ; h2 = same of sq  (full 1027-col scans)
  PE   : S1 = Wmain^T h1(t) + Wlo^T h1(t-1) + Whi^T h1(t+1)   (f32r, per 512 half)
         S2 = same on h2, then += (-441 I)^T m2               (fused mean^2 sub)

The vertical 7-sum halo (3 rows each side) comes from the NEIGHBOR tiles'
h tensors via the Wlo/Whi corner-band matmuls accumulating into the same
PSUM bank, so no input rows are ever re-read or re-scanned. Warm PE runs
f32r matmuls at ~1 cyc/col, making the extra matmuls cheap.
"""

import sys

if "/opt/trn_rl_repo" not in sys.path:
    sys.path.insert(0, "/opt/trn_rl_repo")

import numpy as np
from contextlib import ExitStack

import concourse.bass as bass
import concourse.bacc as bacc
import concourse.tile as tile
from concourse import mybir

H = 1024
W = 1024
C = 3
PER_CORE_B = 2
N_CORES = 8
K7 = 7
PADL, PADR = 7, 3
LW = PADL + W + PADR      # padded lum/sq width (1034)
SCAN_N = W + 3            # h[:, j+3] = centered 7-sum at col j
NTILE = 16                # 2 images x 8 tiles of 128 rows
NRING = 8                 # lum ring depth
NSRING = 5                # sq ring depth
NHRING = 7                # h1/h2 ring depth (read by t-1, t, t+1 matmuls)


def band_weights() -> np.ndarray:
    """Blocks: Wmain | Wlo | Whi | -441*I, each [128,128]."""
    wb = np.zeros((128, 4 * 128), np.float32)
    for m in range(128):
        for k in range(max(m - 3, 0), min(m + 4, 128)):
            wb[k, m] = 1.0
    for m in range(3):  # Wlo: taps from prev tile rows 125+m..127
        for k in range(125 + m, 128):
            wb[k, 128 + m] = 1.0
    for m in range(125, 128):  # Whi: taps from next tile rows 0..m-125
        for k in range(0, m - 124):
            wb[k, 256 + m] = 1.0
    for m in range(128):
        wb[m, 384 + m] = -441.0
    return wb


def build_nc(finalize: bool = True) -> bass.Bass:
    nc = bacc.Bacc("TRN2", target_bir_lowering=False)
    f32 = mybir.dt.float32
    f32r = mybir.dt.float32r

    x = nc.dram_tensor("x", [PER_CORE_B, C, H, W], f32, kind="ExternalInput")
    wbt = nc.dram_tensor("wb", [128, 4 * 128], f32r, kind="ExternalInput")
    y = nc.dram_tensor("y", [PER_CORE_B, 1, H, W], f32, kind="ExternalOutput")

    inv147 = float(np.float32(1.0) / np.float32(147.0))
    inv441 = float(np.float32(1.0) / np.float32(441.0))

    with tile.TileContext(nc) as tc, ExitStack() as ctx:
        cpool = ctx.enter_context(tc.tile_pool(name="const", bufs=1))
        xpool = ctx.enter_context(tc.tile_pool(name="xch", bufs=8))
        mpool = ctx.enter_context(tc.tile_pool(name="m2", bufs=2))
        vpool = ctx.enter_context(tc.tile_pool(name="vout", bufs=2))
        p1pool = ctx.enter_context(tc.tile_pool(name="ps1", bufs=2, space="PSUM"))
        p2pool = ctx.enter_context(tc.tile_pool(name="ps2", bufs=2, space="PSUM"))

        WB = cpool.tile([128, 4 * 128], f32r)
        nc.sync.dma_start(out=WB[:], in_=wbt[:, :])

        # Persistent padded lum/sq rings; zero pads memset once.
        lum_ring = [
            cpool.tile([128, LW], f32, tag=f"lumr{i}", name=f"lumr{i}")
            for i in range(NRING)
        ]
        sq_ring = [
            cpool.tile([128, LW], f32, tag=f"sqr{i}", name=f"sqr{i}")
            for i in range(NSRING)
        ]
        h1_ring = [
            cpool.tile([128, SCAN_N], f32r, tag=f"h1r{i}", name=f"h1r{i}")
            for i in range(NHRING)
        ]
        h2_ring = [
            cpool.tile([128, SCAN_N], f32r, tag=f"h2r{i}", name=f"h2r{i}")
            for i in range(NHRING)
        ]
        for t_ in lum_ring + sq_ring:
            nc.gpsimd.memset(t_[:, 0:PADL], 0.0)
            nc.gpsimd.memset(t_[:, PADL + W : LW], 0.0)

        X1s, X2s, S1s, S2s, M2s, Vs = {}, {}, {}, {}, {}, {}

        def img_row(t):
            return t // 8, 128 * (t % 8)

        RAMP_T = 2   # first tiles skip the high-latency SWDGE accumulate
        SPLIT = 256  # x2-add columns done on DVE; rest on (lockout-stretched) gpsimd

        def st0_load(t):
            b, r0 = img_row(t)
            lum = lum_ring[t % NRING]
            nc.sync.dma_start(
                out=lum[:, PADL : PADL + W], in_=x[b, 0, r0 : r0 + 128, :]
            )
            X2 = xpool.tile([128, W], f32, tag="X2", name=f"X2_{t}")
            nc.sync.dma_start(out=X2[:, :], in_=x[b, 2, r0 : r0 + 128, :])
            X2s[t] = X2
            if t < RAMP_T:
                X1 = xpool.tile([128, W], f32, tag="X1", name=f"X1_{t}")
                nc.sync.dma_start(out=X1[:, :], in_=x[b, 1, r0 : r0 + 128, :])
                X1s[t] = X1

        def st1a_add(t):
            # x1 folded in during the DMA itself (SWDGE accumulate)
            b, r0 = img_row(t)
            lum = lum_ring[t % NRING]
            if t < RAMP_T:
                nc.vector.tensor_add(
                    lum[:, PADL : PADL + W],
                    lum[:, PADL : PADL + W],
                    X1s.pop(t)[:, :],
                )
            else:
                nc.gpsimd.dma_start(
                    out=lum[:, PADL : PADL + W],
                    in_=x[b, 1, r0 : r0 + 128, :],
                    accum_op=mybir.AluOpType.add,
                )

        def st1b_add(t):
            # x2 add on DVE: keeps the whole pointwise pool on one engine,
            # avoiding the DVE<->gpsimd SBUF-lockout lockstep.
            lum = lum_ring[t % NRING]
            nc.vector.tensor_add(
                lum[:, PADL : PADL + W],
                lum[:, PADL : PADL + W],
                X2s.pop(t)[:, :],
            )

        def st2_sq(t):
            lum = lum_ring[t % NRING]
            sq = sq_ring[t % NSRING]
            nc.scalar.activation(
                sq[:, PADL : PADL + W],
                lum[:, PADL : PADL + W],
                mybir.ActivationFunctionType.Square,
            )

        def st3_scan(t):
            lum = lum_ring[t % NRING]
            sq = sq_ring[t % NSRING]
            h1 = h1_ring[t % NHRING]
            h2 = h2_ring[t % NHRING]
            # h[:, j] = sum src[j-6..j]; matmul reads h[:, n+3] = centered sum
            for h, src in ((h1, lum), (h2, sq)):
                nc.vector.tensor_tensor_scan(
                    out=h[:, 0:SCAN_N],
                    data0=src[:, PADL : PADL + SCAN_N],
                    data1=src[:, 0:SCAN_N],
                    initial=0.0,
                    op0=mybir.AluOpType.add,
                    op1=mybir.AluOpType.subtract,
                )

        def _banded(S, hring, t, extra=None):
            """S[:, half] = Wmain^T h(t) + Wlo^T h(t-1) + Whi^T h(t+1) [+ extra]."""
            ti = t % 8
            for hf in range(2):
                lo, hi = 3 + 512 * hf, 3 + 512 * hf + 512
                parts = [(0, hring[t % NHRING])]
                if ti > 0:
                    parts.append((1, hring[(t - 1) % NHRING]))
                if ti < 7:
                    parts.append((2, hring[(t + 1) % NHRING]))
                n = len(parts) + (1 if extra is not None else 0)
                for j, (wsel, hsrc) in enumerate(parts):
                    nc.tensor.matmul(
                        S[0:128, 512 * hf : 512 * hf + 512],
                        WB[0:128, 128 * wsel : 128 * wsel + 128],
                        hsrc[0:128, lo:hi],
                        start=(j == 0),
                        stop=(j == n - 1),
                    )
                if extra is not None:
                    nc.tensor.matmul(
                        S[0:128, 512 * hf : 512 * hf + 512],
                        WB[0:128, 384:512],
                        extra[0:128, 512 * hf : 512 * hf + 512],
                        start=False,
                        stop=True,
                    )

        def st4_mm1(t):
            S1 = p1pool.tile([128, W], f32, tag="S1", name=f"S1_{t}")
            _banded(S1, h1_ring, t)
            S1s[t] = S1

        def st5_m2(t):
            m2 = mpool.tile([128, W], f32r, tag="m2", name=f"m2_{t}")
            nc.scalar.activation(
                m2[:, :],
                S1s.pop(t)[:, :],
                mybir.ActivationFunctionType.Square,
                scale=inv147,
            )
            M2s[t] = m2

        def st6_mm2(t):
            S2 = p2pool.tile([128, W], f32, tag="S2", name=f"S2_{t}")
            _banded(S2, h2_ring, t, extra=M2s.pop(t))
            S2s[t] = S2

        def st7_v(t):
            V = vpool.tile([128, W], f32, tag="V", name=f"V_{t}")
            nc.scalar.activation(
                V[:, :],
                S2s.pop(t)[:, :],
                mybir.ActivationFunctionType.Copy,
                scale=inv441,
            )
            Vs[t] = V

        def st8_out(t):
            # ACT-ring HWDGE right after ACT's own V copy: the trigger's
            # data dependency is already satisfied in-order, so it never
            # stalls the scalar queue (unlike a cross-engine wait would).
            b, r0 = img_row(t)
            nc.scalar.dma_start(
                out=y[b, 0, r0 : r0 + 128, :], in_=Vs.pop(t)[0:128, :]
            )

        # Software-pipelined emission with ~2 iterations of slack per stage.
        # Stage s of tile i-s runs in iteration i. On DVE the early-stage
        # x1-add is emitted before the older tile's scans (no head-blocking).
        for i in range(NTILE + 9):
            if i < NTILE:
                st0_load(i)
            if 1 <= i < NTILE + 1:
                st1a_add(i - 1)
            if 2 <= i < NTILE + 2:
                st1b_add(i - 2)
            if 3 <= i < NTILE + 3:
                st2_sq(i - 3)
            if 4 <= i < NTILE + 4:
                st3_scan(i - 4)
            if 6 <= i < NTILE + 6:
                st4_mm1(i - 6)
            if 7 <= i < NTILE + 7:
                st5_m2(i - 7)
            if 8 <= i < NTILE + 8:
                st6_mm2(i - 8)
            if 9 <= i < NTILE + 9:
                st7_v(i - 9)
                st8_out(i - 9)

    if finalize:
        nc.finalize()
    return nc


def kernel(x, kernel_size):
    assert int(kernel_size) == K7
    x = np.ascontiguousarray(np.asarray(x, dtype=np.float32))
    B = x.shape[0]
    assert x.shape == (B, C, H, W) and B == PER_CORE_B * N_CORES

    from concourse.bass_utils import run_bass_kernel_spmd

    nc = build_nc()
    wb = band_weights()
    in_maps = [
        {"x": x[i * PER_CORE_B : (i + 1) * PER_CORE_B], "wb": wb}
        for i in range(N_CORES)
    ]
    res = run_bass_kernel_spmd(nc, in_maps, list(range(N_CORES)))
    y = np.concatenate([res.results[i]["y"] for i in range(N_CORES)], axis=0)
    return y


# revision 29
# speedup vs baseline: 1.0652x; 1.0002x over previous
"""LocalVarianceMap Trainium2 kernel (v2: halo-free tiling, warm PE).

reference:
  lum  = mean over channel of x            (B,1,H,W)
  mean = 7x7 'same' box mean of lum ; sqm = same of lum^2
  out  = sqm - mean^2

Full input x: (16, 3, 1024, 1024) fp32. Data-parallel over batch:
8 NeuronCores x 2 images each.

Per-core: 16 non-overlapping 128-row tiles (2 images x 8). Per tile:
  sync : load x0 -> lum (padded ring), x1 -> X1, x2 -> X2; store V -> y
  gpsimd: lum += X1 ; lum += X2            (channel sum, 2 TT adds)
  ACT  : sq = Square(lum) ; m2 = Square(S1/147) ; V = Copy(S2/441)
  DVE  : h1 = width-7 box scan of lum ; h2 = same of sq  (full 1027-col scans)
  PE   : S1 = Wmain^T h1(t) + Wlo^T h1(t-1) + Whi^T h1(t+1)   (f32r, per 512 half)
         S2 = same on h2, then += (-441 I)^T m2               (fused mean^2 sub)

The vertical 7-sum halo (3 rows each side) comes from the NEIGHBOR tiles'
h tensors via the Wlo/Whi corner-band matmuls accumulating into the same
PSUM bank, so no input rows are ever re-read or re-scanned. Warm PE runs
f32r matmuls at ~1 cyc/col, making the extra matmuls cheap.
"""

import sys

if "/opt/trn_rl_repo" not in sys.path:
    sys.path.insert(0, "/opt/trn_rl_repo")

import numpy as np
from contextlib import ExitStack

import concourse.bass as bass
import concourse.bacc as bacc
import concourse.tile as tile
from concourse import mybir

H = 1024
W = 1024
C = 3
PER_CORE_B = 2
N_CORES = 8
K7 = 7
PADL, PADR = 7, 3
LW = PADL + W + PADR      # padded lum/sq width (1034)
SCAN_N = W + 3            # h[:, j+3] = centered 7-sum at col j
NTILE = 16                # 2 images x 8 tiles of 128 rows
NRING = 8                 # lum ring depth
NSRING = 5                # sq ring depth
NHRING = 7                # h1/h2 ring depth (read by t-1, t, t+1 matmuls)


def band_weights() -> np.ndarray:
    """Blocks: Wmain | Wlo | Whi | -441*I, each [128,128]."""
    wb = np.zeros((128, 4 * 128), np.float32)
    for m in range(128):
        for k in range(max(m - 3, 0), min(m + 4, 128)):
            wb[k, m] = 1.0
    for m in range(3):  # Wlo: taps from prev tile rows 125+m..127
        for k in range(125 + m, 128):
            wb[k, 128 + m] = 1.0
    for m in range(125, 128):  # Whi: taps from next tile rows 0..m-125
        for k in range(0, m - 124):
            wb[k, 256 + m] = 1.0
    for m in range(128):
        wb[m, 384 + m] = -441.0
    return wb


def build_nc(finalize: bool = True) -> bass.Bass:
    nc = bacc.Bacc("TRN2", target_bir_lowering=False)
    f32 = mybir.dt.float32
    f32r = mybir.dt.float32r

    x = nc.dram_tensor("x", [PER_CORE_B, C, H, W], f32, kind="ExternalInput")
    wbt = nc.dram_tensor("wb", [128, 4 * 128], f32r, kind="ExternalInput")
    y = nc.dram_tensor("y", [PER_CORE_B, 1, H, W], f32, kind="ExternalOutput")

    inv147 = float(np.float32(1.0) / np.float32(147.0))
    inv441 = float(np.float32(1.0) / np.float32(441.0))

    with tile.TileContext(nc) as tc, ExitStack() as ctx:
        cpool = ctx.enter_context(tc.tile_pool(name="const", bufs=1))
        xpool = ctx.enter_context(tc.tile_pool(name="xch", bufs=8))
        mpool = ctx.enter_context(tc.tile_pool(name="m2", bufs=2))
        vpool = ctx.enter_context(tc.tile_pool(name="vout", bufs=2))
        p1pool = ctx.enter_context(tc.tile_pool(name="ps1", bufs=2, space="PSUM"))
        p2pool = ctx.enter_context(tc.tile_pool(name="ps2", bufs=2, space="PSUM"))

        WB = cpool.tile([128, 4 * 128], f32r)
        nc.sync.dma_start(out=WB[:], in_=wbt[:, :])

        # Persistent padded lum/sq rings; zero pads memset once.
        lum_ring = [
            cpool.tile([128, LW], f32, tag=f"lumr{i}", name=f"lumr{i}")
            for i in range(NRING)
        ]
        sq_ring = [
            cpool.tile([128, LW], f32, tag=f"sqr{i}", name=f"sqr{i}")
            for i in range(NSRING)
        ]
        h1_ring = [
            cpool.tile([128, SCAN_N], f32r, tag=f"h1r{i}", name=f"h1r{i}")
            for i in range(NHRING)
        ]
        h2_ring = [
            cpool.tile([128, SCAN_N], f32r, tag=f"h2r{i}", name=f"h2r{i}")
            for i in range(NHRING)
        ]
        for t_ in lum_ring + sq_ring:
            nc.gpsimd.memset(t_[:, 0:PADL], 0.0)
            nc.gpsimd.memset(t_[:, PADL + W : LW], 0.0)

        X1s, X2s, S1s, S2s, M2s, Vs = {}, {}, {}, {}, {}, {}

        def img_row(t):
            return t // 8, 128 * (t % 8)

        RAMP_T = 2   # first tiles skip the high-latency SWDGE accumulate
        SPLIT = 256  # x2-add columns done on DVE; rest on (lockout-stretched) gpsimd

        def st0_load(t):
            b, r0 = img_row(t)
            lum = lum_ring[t % NRING]
            nc.sync.dma_start(
                out=lum[:, PADL : PADL + W], in_=x[b, 0, r0 : r0 + 128, :]
            )
            X2 = xpool.tile([128, W], f32, tag="X2", name=f"X2_{t}")
            nc.sync.dma_start(out=X2[:, :], in_=x[b, 2, r0 : r0 + 128, :])
            X2s[t] = X2
            if t < RAMP_T:
                X1 = xpool.tile([128, W], f32, tag="X1", name=f"X1_{t}")
                nc.sync.dma_start(out=X1[:, :], in_=x[b, 1, r0 : r0 + 128, :])
                X1s[t] = X1
            else:
                # x1 folded in during the DMA itself (SWDGE accumulate).
                # Triggered at load-emission time: the trigger's semaphore
                # wait on the x0 load makes it fire as soon as x0 lands,
                # maximizing slack before the DVE needs lum.
                nc.gpsimd.dma_start(
                    out=lum[:, PADL : PADL + W],
                    in_=x[b, 1, r0 : r0 + 128, :],
                    accum_op=mybir.AluOpType.add,
                )

        def st1a_add(t):
            if t < RAMP_T:
                lum = lum_ring[t % NRING]
                nc.vector.tensor_add(
                    lum[:, PADL : PADL + W],
                    lum[:, PADL : PADL + W],
                    X1s.pop(t)[:, :],
                )

        def st1b_add(t):
            # x2 add on DVE: keeps the whole pointwise pool on one engine,
            # avoiding the DVE<->gpsimd SBUF-lockout lockstep.
            lum = lum_ring[t % NRING]
            nc.vector.tensor_add(
                lum[:, PADL : PADL + W],
                lum[:, PADL : PADL + W],
                X2s.pop(t)[:, :],
            )

        def st2_sq(t):
            lum = lum_ring[t % NRING]
            sq = sq_ring[t % NSRING]
            nc.scalar.activation(
                sq[:, PADL : PADL + W],
                lum[:, PADL : PADL + W],
                mybir.ActivationFunctionType.Square,
            )

        def st3_scan(t):
            lum = lum_ring[t % NRING]
            sq = sq_ring[t % NSRING]
            h1 = h1_ring[t % NHRING]
            h2 = h2_ring[t % NHRING]
            # h[:, j] = sum src[j-6..j]; matmul reads h[:, n+3] = centered sum
            for h, src in ((h1, lum), (h2, sq)):
                nc.vector.tensor_tensor_scan(
                    out=h[:, 0:SCAN_N],
                    data0=src[:, PADL : PADL + SCAN_N],
                    data1=src[:, 0:SCAN_N],
                    initial=0.0,
                    op0=mybir.AluOpType.add,
                    op1=mybir.AluOpType.subtract,
                )

        def _banded(S, hring, t, extra=None):
            """S[:, half] = Wmain^T h(t) + Wlo^T h(t-1) + Whi^T h(t+1) [+ extra]."""
            ti = t % 8
            for hf in range(2):
                lo, hi = 3 + 512 * hf, 3 + 512 * hf + 512
                parts = [(0, hring[t % NHRING])]
                if ti > 0:
                    parts.append((1, hring[(t - 1) % NHRING]))
                if ti < 7:
                    parts.append((2, hring[(t + 1) % NHRING]))
                n = len(parts) + (1 if extra is not None else 0)
                for j, (wsel, hsrc) in enumerate(parts):
                    nc.tensor.matmul(
                        S[0:128, 512 * hf : 512 * hf + 512],
                        WB[0:128, 128 * wsel : 128 * wsel + 128],
                        hsrc[0:128, lo:hi],
                        start=(j == 0),
                        stop=(j == n - 1),
                    )
                if extra is not None:
                    nc.tensor.matmul(
                        S[0:128, 512 * hf : 512 * hf + 512],
                        WB[0:128, 384:512],
                        extra[0:128, 512 * hf : 512 * hf + 512],
                        start=False,
                        stop=True,
                    )

        def st4_mm1(t):
            S1 = p1pool.tile([128, W], f32, tag="S1", name=f"S1_{t}")
            _banded(S1, h1_ring, t)
            S1s[t] = S1

        def st5_m2(t):
            m2 = mpool.tile([128, W], f32r, tag="m2", name=f"m2_{t}")
            nc.scalar.activation(
                m2[:, :],
                S1s.pop(t)[:, :],
                mybir.ActivationFunctionType.Square,
                scale=inv147,
            )
            M2s[t] = m2

        def st6_mm2(t):
            S2 = p2pool.tile([128, W], f32, tag="S2", name=f"S2_{t}")
            _banded(S2, h2_ring, t, extra=M2s.pop(t))
            S2s[t] = S2

        def st7_v(t):
            V = vpool.tile([128, W], f32, tag="V", name=f"V_{t}")
            nc.scalar.activation(
                V[:, :],
                S2s.pop(t)[:, :],
                mybir.ActivationFunctionType.Copy,
                scale=inv441,
            )
            Vs[t] = V

        def st8_out(t):
            # ACT-ring HWDGE right after ACT's own V copy: the trigger's
            # data dependency is already satisfied in-order, so it never
            # stalls the scalar queue (unlike a cross-engine wait would).
            b, r0 = img_row(t)
            nc.scalar.dma_start(
                out=y[b, 0, r0 : r0 + 128, :], in_=Vs.pop(t)[0:128, :]
            )

        # Software-pipelined emission with ~2 iterations of slack per stage.
        # Stage s of tile i-s runs in iteration i. On DVE the early-stage
        # x1-add is emitted before the older tile's scans (no head-blocking).
        for i in range(NTILE + 9):
            if i < NTILE:
                st0_load(i)
            if 1 <= i < NTILE + 1:
                st1a_add(i - 1)
            if 2 <= i < NTILE + 2:
                st1b_add(i - 2)
            if 3 <= i < NTILE + 3:
                st2_sq(i - 3)
            if 4 <= i < NTILE + 4:
                st3_scan(i - 4)
            if 6 <= i < NTILE + 6:
                st4_mm1(i - 6)
            if 7 <= i < NTILE + 7:
                st5_m2(i - 7)
            if 8 <= i < NTILE + 8:
                st6_mm2(i - 8)
            if 9 <= i < NTILE + 9:
                st7_v(i - 9)
                st8_out(i - 9)

    if finalize:
        nc.finalize()
    return nc


def kernel(x, kernel_size):
    assert int(kernel_size) == K7
    x = np.ascontiguousarray(np.asarray(x, dtype=np.float32))
    B = x.shape[0]
    assert x.shape == (B, C, H, W) and B == PER_CORE_B * N_CORES

    from concourse.bass_utils import run_bass_kernel_spmd

    nc = build_nc()
    wb = band_weights()
    in_maps = [
        {"x": x[i * PER_CORE_B : (i + 1) * PER_CORE_B], "wb": wb}
        for i in range(N_CORES)
    ]
    res = run_bass_kernel_spmd(nc, in_maps, list(range(N_CORES)))
    y = np.concatenate([res.results[i]["y"] for i in range(N_CORES)], axis=0)
    return y


# revision 31
# speedup vs baseline: 1.0731x; 1.0074x over previous
"""LocalVarianceMap Trainium2 kernel (v2: halo-free tiling, warm PE).

reference:
  lum  = mean over channel of x            (B,1,H,W)
  mean = 7x7 'same' box mean of lum ; sqm = same of lum^2
  out  = sqm - mean^2

Full input x: (16, 3, 1024, 1024) fp32. Data-parallel over batch:
8 NeuronCores x 2 images each.

Per-core: 16 non-overlapping 128-row tiles (2 images x 8). Per tile:
  sync : load x0 -> lum (padded ring), x1 -> X1, x2 -> X2; store V -> y
  gpsimd: lum += X1 ; lum += X2            (channel sum, 2 TT adds)
  ACT  : sq = Square(lum) ; m2 = Square(S1/147) ; V = Copy(S2/441)
  DVE  : h1 = width-7 box scan of lum ; h2 = same of sq  (full 1027-col scans)
  PE   : S1 = Wmain^T h1(t) + Wlo^T h1(t-1) + Whi^T h1(t+1)   (f32r, per 512 half)
         S2 = same on h2, then += (-441 I)^T m2               (fused mean^2 sub)

The vertical 7-sum halo (3 rows each side) comes from the NEIGHBOR tiles'
h tensors via the Wlo/Whi corner-band matmuls accumulating into the same
PSUM bank, so no input rows are ever re-read or re-scanned. Warm PE runs
f32r matmuls at ~1 cyc/col, making the extra matmuls cheap.
"""

import sys

if "/opt/trn_rl_repo" not in sys.path:
    sys.path.insert(0, "/opt/trn_rl_repo")

import numpy as np
from contextlib import ExitStack

import concourse.bass as bass
import concourse.bacc as bacc
import concourse.tile as tile
from concourse import mybir

H = 1024
W = 1024
C = 3
PER_CORE_B = 2
N_CORES = 8
K7 = 7
PADL, PADR = 7, 3
LW = PADL + W + PADR      # padded lum/sq width (1034)
SCAN_N = W + 3            # h[:, j+3] = centered 7-sum at col j
NTILE = 16                # 2 images x 8 tiles of 128 rows
NRING = 8                 # lum ring depth
NSRING = 5                # sq ring depth
NHRING = 7                # h1/h2 ring depth (read by t-1, t, t+1 matmuls)


def band_weights() -> np.ndarray:
    """Blocks: Wmain | Wlo | Whi | -441*I, each [128,128]."""
    wb = np.zeros((128, 4 * 128), np.float32)
    for m in range(128):
        for k in range(max(m - 3, 0), min(m + 4, 128)):
            wb[k, m] = 1.0
    for m in range(3):  # Wlo: taps from prev tile rows 125+m..127
        for k in range(125 + m, 128):
            wb[k, 128 + m] = 1.0
    for m in range(125, 128):  # Whi: taps from next tile rows 0..m-125
        for k in range(0, m - 124):
            wb[k, 256 + m] = 1.0
    for m in range(128):
        wb[m, 384 + m] = -441.0
    return wb


def build_nc(finalize: bool = True) -> bass.Bass:
    nc = bacc.Bacc("TRN2", target_bir_lowering=False)
    f32 = mybir.dt.float32
    f32r = mybir.dt.float32r

    x = nc.dram_tensor("x", [PER_CORE_B, C, H, W], f32, kind="ExternalInput")
    wbt = nc.dram_tensor("wb", [128, 4 * 128], f32r, kind="ExternalInput")
    y = nc.dram_tensor("y", [PER_CORE_B, 1, H, W], f32, kind="ExternalOutput")

    inv147 = float(np.float32(1.0) / np.float32(147.0))
    inv441 = float(np.float32(1.0) / np.float32(441.0))

    with tile.TileContext(nc) as tc, ExitStack() as ctx:
        cpool = ctx.enter_context(tc.tile_pool(name="const", bufs=1))
        xpool = ctx.enter_context(tc.tile_pool(name="xch", bufs=10))
        mpool = ctx.enter_context(tc.tile_pool(name="m2", bufs=2))
        vpool = ctx.enter_context(tc.tile_pool(name="vout", bufs=2))
        p1pool = ctx.enter_context(tc.tile_pool(name="ps1", bufs=2, space="PSUM"))
        p2pool = ctx.enter_context(tc.tile_pool(name="ps2", bufs=2, space="PSUM"))

        WB = cpool.tile([128, 4 * 128], f32r)
        nc.sync.dma_start(out=WB[:], in_=wbt[:, :])

        # Persistent padded lum/sq rings; zero pads memset once.
        lum_ring = [
            cpool.tile([128, LW], f32, tag=f"lumr{i}", name=f"lumr{i}")
            for i in range(NRING)
        ]
        sq_ring = [
            cpool.tile([128, LW], f32, tag=f"sqr{i}", name=f"sqr{i}")
            for i in range(NSRING)
        ]
        h1_ring = [
            cpool.tile([128, SCAN_N], f32r, tag=f"h1r{i}", name=f"h1r{i}")
            for i in range(NHRING)
        ]
        h2_ring = [
            cpool.tile([128, SCAN_N], f32r, tag=f"h2r{i}", name=f"h2r{i}")
            for i in range(NHRING)
        ]
        for t_ in lum_ring + sq_ring:
            nc.gpsimd.memset(t_[:, 0:PADL], 0.0)
            nc.gpsimd.memset(t_[:, PADL + W : LW], 0.0)

        X1s, X2s, S1s, S2s, M2s, Vs = {}, {}, {}, {}, {}, {}

        def img_row(t):
            return t // 8, 128 * (t % 8)

        RAMP_T = 4   # first tiles skip the high-latency SWDGE accumulate

        def st0_load(t):
            b, r0 = img_row(t)
            lum = lum_ring[t % NRING]
            nc.sync.dma_start(
                out=lum[:, PADL : PADL + W], in_=x[b, 0, r0 : r0 + 128, :]
            )
            X2 = xpool.tile([128, W], f32, tag="X2", name=f"X2_{t}")
            nc.sync.dma_start(out=X2[:, :], in_=x[b, 2, r0 : r0 + 128, :])
            X2s[t] = X2
            if t < RAMP_T:
                X1 = xpool.tile([128, W], f32, tag="X1", name=f"X1_{t}")
                nc.sync.dma_start(out=X1[:, :], in_=x[b, 1, r0 : r0 + 128, :])
                X1s[t] = X1
            else:
                # x1 folded in during the DMA itself (SWDGE accumulate).
                # Triggered at load-emission time: the trigger's semaphore
                # wait on the x0 load makes it fire as soon as x0 lands,
                # maximizing slack before the DVE needs lum.
                nc.gpsimd.dma_start(
                    out=lum[:, PADL : PADL + W],
                    in_=x[b, 1, r0 : r0 + 128, :],
                    accum_op=mybir.AluOpType.add,
                )

        def st1a_add(t):
            if t < RAMP_T:
                lum = lum_ring[t % NRING]
                nc.vector.tensor_add(
                    lum[:, PADL : PADL + W],
                    lum[:, PADL : PADL + W],
                    X1s.pop(t)[:, :],
                )

        def st1b_add(t):
            # x2 add on DVE: keeps the whole pointwise pool on one engine,
            # avoiding the DVE<->gpsimd SBUF-lockout lockstep.
            lum = lum_ring[t % NRING]
            nc.vector.tensor_add(
                lum[:, PADL : PADL + W],
                lum[:, PADL : PADL + W],
                X2s.pop(t)[:, :],
            )

        def st2_sq(t):
            lum = lum_ring[t % NRING]
            sq = sq_ring[t % NSRING]
            nc.scalar.activation(
                sq[:, PADL : PADL + W],
                lum[:, PADL : PADL + W],
                mybir.ActivationFunctionType.Square,
            )

        def st3_scan(t):
            lum = lum_ring[t % NRING]
            sq = sq_ring[t % NSRING]
            h1 = h1_ring[t % NHRING]
            h2 = h2_ring[t % NHRING]
            # h[:, j] = sum src[j-6..j]; matmul reads h[:, n+3] = centered sum
            for h, src in ((h1, lum), (h2, sq)):
                nc.vector.tensor_tensor_scan(
                    out=h[:, 0:SCAN_N],
                    data0=src[:, PADL : PADL + SCAN_N],
                    data1=src[:, 0:SCAN_N],
                    initial=0.0,
                    op0=mybir.AluOpType.add,
                    op1=mybir.AluOpType.subtract,
                )

        def _banded(S, hring, t, extra=None):
            """S[:, half] = Wmain^T h(t) + Wlo^T h(t-1) + Whi^T h(t+1) [+ extra]."""
            ti = t % 8
            for hf in range(2):
                lo, hi = 3 + 512 * hf, 3 + 512 * hf + 512
                parts = [(0, hring[t % NHRING])]
                if ti > 0:
                    parts.append((1, hring[(t - 1) % NHRING]))
                if ti < 7:
                    parts.append((2, hring[(t + 1) % NHRING]))
                n = len(parts) + (1 if extra is not None else 0)
                for j, (wsel, hsrc) in enumerate(parts):
                    nc.tensor.matmul(
                        S[0:128, 512 * hf : 512 * hf + 512],
                        WB[0:128, 128 * wsel : 128 * wsel + 128],
                        hsrc[0:128, lo:hi],
                        start=(j == 0),
                        stop=(j == n - 1),
                    )
                if extra is not None:
                    nc.tensor.matmul(
                        S[0:128, 512 * hf : 512 * hf + 512],
                        WB[0:128, 384:512],
                        extra[0:128, 512 * hf : 512 * hf + 512],
                        start=False,
                        stop=True,
                    )

        def st4_mm1(t):
            S1 = p1pool.tile([128, W], f32, tag="S1", name=f"S1_{t}")
            _banded(S1, h1_ring, t)
            S1s[t] = S1

        def st5_m2(t):
            m2 = mpool.tile([128, W], f32r, tag="m2", name=f"m2_{t}")
            nc.scalar.activation(
                m2[:, :],
                S1s.pop(t)[:, :],
                mybir.ActivationFunctionType.Square,
                scale=inv147,
            )
            M2s[t] = m2

        def st6_mm2(t):
            S2 = p2pool.tile([128, W], f32, tag="S2", name=f"S2_{t}")
            _banded(S2, h2_ring, t, extra=M2s.pop(t))
            S2s[t] = S2

        def st7_v(t):
            V = vpool.tile([128, W], f32, tag="V", name=f"V_{t}")
            nc.scalar.activation(
                V[:, :],
                S2s.pop(t)[:, :],
                mybir.ActivationFunctionType.Copy,
                scale=inv441,
            )
            Vs[t] = V

        def st8_out(t):
            # ACT-ring HWDGE right after ACT's own V copy: the trigger's
            # data dependency is already satisfied in-order, so it never
            # stalls the scalar queue (unlike a cross-engine wait would).
            b, r0 = img_row(t)
            nc.scalar.dma_start(
                out=y[b, 0, r0 : r0 + 128, :], in_=Vs.pop(t)[0:128, :]
            )

        # Software-pipelined emission with ~2 iterations of slack per stage.
        # Stage s of tile i-s runs in iteration i. On DVE the early-stage
        # x1-add is emitted before the older tile's scans (no head-blocking).
        for i in range(NTILE + 9):
            if i < NTILE:
                st0_load(i)
            if 1 <= i < NTILE + 1:
                st1a_add(i - 1)
            if 2 <= i < NTILE + 2:
                st1b_add(i - 2)
            if 3 <= i < NTILE + 3:
                st2_sq(i - 3)
            if 4 <= i < NTILE + 4:
                st3_scan(i - 4)
            if 6 <= i < NTILE + 6:
                st4_mm1(i - 6)
            if 7 <= i < NTILE + 7:
                st5_m2(i - 7)
            if 8 <= i < NTILE + 8:
                st6_mm2(i - 8)
            if 9 <= i < NTILE + 9:
                st7_v(i - 9)
                st8_out(i - 9)

    if finalize:
        nc.finalize()
    return nc


def kernel(x, kernel_size):
    assert int(kernel_size) == K7
    x = np.ascontiguousarray(np.asarray(x, dtype=np.float32))
    B = x.shape[0]
    assert x.shape == (B, C, H, W) and B == PER_CORE_B * N_CORES

    from concourse.bass_utils import run_bass_kernel_spmd

    nc = build_nc()
    wb = band_weights()
    in_maps = [
        {"x": x[i * PER_CORE_B : (i + 1) * PER_CORE_B], "wb": wb}
        for i in range(N_CORES)
    ]
    res = run_bass_kernel_spmd(nc, in_maps, list(range(N_CORES)))
    y = np.concatenate([res.results[i]["y"] for i in range(N_CORES)], axis=0)
    return y


# revision 32
# speedup vs baseline: 1.0960x; 1.0213x over previous
"""LocalVarianceMap Trainium2 kernel (halo-free tiling, SDMA-roofline paced).

reference:
  lum  = mean over channel of x            (B,1,H,W)
  mean = 7x7 'same' box mean of lum ; sqm = same of lum^2
  out  = sqm - mean^2

Full input x: (16, 3, 1024, 1024) fp32. Data-parallel over batch:
8 NeuronCores x 2 images each.

Per-core: 16 non-overlapping 128-row tiles (2 images x 8). Per tile:
  sync  : load x0 -> lum (padded ring), x2 -> X2 pool
  gpsimd: SWDGE accumulate-DMA x1 += lum (triggered at load-emission time so
          it fires the moment x0 lands; a DMA-side add costs no engine time)
  DVE   : lum += X2 (TT add); h1/h2 = width-7 box scans of lum and sq
          (single full-width 1027-col scans, ~2.3us each)
  ACT   : sq = Square(lum) ; m2 = Square(S1/147) ; V = Copy(S2/441);
          out-DMA trigger on the ACT HWDGE ring right after its own V copy
          (in-order => the trigger never stalls the queue)
  PE    : S1 = Wmain^T h1(t) + Wlo^T h1(t-1) + Whi^T h1(t+1)  (f32r, 512 halves)
          S2 = same on h2, then += (-441 I)^T m2              (fused mean^2 sub)

Key design facts (measured on HW):
- The vertical 7-sum halo (3 rows each side) comes from the NEIGHBOR tiles'
  h tensors via the Wlo/Whi corner-band matmuls accumulating into the same
  PSUM bank, so no input rows are ever re-read or re-scanned; input traffic
  is the 33.5 MiB/core minimum.
- f32r matmuls stream at ~1 cyc/col once the PE p-state is warm; the corner
  matmuls are cheap. fp32(non-r) matmuls would be 3x slower (LOW_HIGH 2-pass).
- GPSIMD tensor ops run ~2x slower whenever the DVE is busy (SBUF lockout),
  so the x2 add lives on the DVE; pointwise work on gpsimd would lockstep
  with the scans and pace the whole kernel.
- The x1 accumulate costs ~15.5 GB/s/engine (SBUF RMW) vs 22.3 plain, an
  acceptable tax vs +1.2us/tile of DVE time.
- First RAMP_T tiles use plain x1 loads + DVE adds to skip the ~4-5us SWDGE
  accumulate latency during pipeline ramp.
Steady state is paced by the SDMA at ~6.4us/tile; DVE is ~5.8.
"""

import sys

if "/opt/trn_rl_repo" not in sys.path:
    sys.path.insert(0, "/opt/trn_rl_repo")

import numpy as np
from contextlib import ExitStack

import concourse.bass as bass
import concourse.bacc as bacc
import concourse.tile as tile
from concourse import mybir

H = 1024
W = 1024
C = 3
PER_CORE_B = 2
N_CORES = 8
K7 = 7
PADL, PADR = 7, 3
LW = PADL + W + PADR      # padded lum/sq width (1034)
SCAN_N = W + 3            # h[:, j+3] = centered 7-sum at col j
NTILE = 16                # 2 images x 8 tiles of 128 rows
NRING = 8                 # lum ring depth
NSRING = 5                # sq ring depth
NHRING = 7                # h1/h2 ring depth (read by t-1, t, t+1 matmuls)


def band_weights() -> np.ndarray:
    """Blocks: Wmain | Wlo | Whi | -441*I, each [128,128]."""
    wb = np.zeros((128, 4 * 128), np.float32)
    for m in range(128):
        for k in range(max(m - 3, 0), min(m + 4, 128)):
            wb[k, m] = 1.0
    for m in range(3):  # Wlo: taps from prev tile rows 125+m..127
        for k in range(125 + m, 128):
            wb[k, 128 + m] = 1.0
    for m in range(125, 128):  # Whi: taps from next tile rows 0..m-125
        for k in range(0, m - 124):
            wb[k, 256 + m] = 1.0
    for m in range(128):
        wb[m, 384 + m] = -441.0
    return wb


def build_nc(finalize: bool = True) -> bass.Bass:
    nc = bacc.Bacc("TRN2", target_bir_lowering=False)
    f32 = mybir.dt.float32
    f32r = mybir.dt.float32r

    x = nc.dram_tensor("x", [PER_CORE_B, C, H, W], f32, kind="ExternalInput")
    wbt = nc.dram_tensor("wb", [128, 4 * 128], f32r, kind="ExternalInput")
    y = nc.dram_tensor("y", [PER_CORE_B, 1, H, W], f32, kind="ExternalOutput")

    inv147 = float(np.float32(1.0) / np.float32(147.0))
    inv441 = float(np.float32(1.0) / np.float32(441.0))

    with tile.TileContext(nc) as tc, ExitStack() as ctx:
        cpool = ctx.enter_context(tc.tile_pool(name="const", bufs=1))
        xpool = ctx.enter_context(tc.tile_pool(name="xch", bufs=10))
        mpool = ctx.enter_context(tc.tile_pool(name="m2", bufs=2))
        vpool = ctx.enter_context(tc.tile_pool(name="vout", bufs=2))
        p1pool = ctx.enter_context(tc.tile_pool(name="ps1", bufs=2, space="PSUM"))
        p2pool = ctx.enter_context(tc.tile_pool(name="ps2", bufs=2, space="PSUM"))

        WB = cpool.tile([128, 4 * 128], f32r)
        nc.sync.dma_start(out=WB[:], in_=wbt[:, :])

        # Persistent padded lum/sq rings; zero pads memset once.
        lum_ring = [
            cpool.tile([128, LW], f32, tag=f"lumr{i}", name=f"lumr{i}")
            for i in range(NRING)
        ]
        sq_ring = [
            cpool.tile([128, LW], f32, tag=f"sqr{i}", name=f"sqr{i}")
            for i in range(NSRING)
        ]
        h1_ring = [
            cpool.tile([128, SCAN_N], f32r, tag=f"h1r{i}", name=f"h1r{i}")
            for i in range(NHRING)
        ]
        h2_ring = [
            cpool.tile([128, SCAN_N], f32r, tag=f"h2r{i}", name=f"h2r{i}")
            for i in range(NHRING)
        ]
        for t_ in lum_ring + sq_ring:
            nc.gpsimd.memset(t_[:, 0:PADL], 0.0)
            nc.gpsimd.memset(t_[:, PADL + W : LW], 0.0)

        X1s, X2s, S1s, S2s, M2s, Vs = {}, {}, {}, {}, {}, {}

        def img_row(t):
            return t // 8, 128 * (t % 8)

        RAMP_T = 4   # first tiles skip the high-latency SWDGE accumulate

        def st0_load(t):
            b, r0 = img_row(t)
            lum = lum_ring[t % NRING]
            nc.sync.dma_start(
                out=lum[:, PADL : PADL + W], in_=x[b, 0, r0 : r0 + 128, :]
            )
            X2 = xpool.tile([128, W], f32, tag="X2", name=f"X2_{t}")
            nc.sync.dma_start(out=X2[:, :], in_=x[b, 2, r0 : r0 + 128, :])
            X2s[t] = X2
            if t < RAMP_T:
                X1 = xpool.tile([128, W], f32, tag="X1", name=f"X1_{t}")
                nc.sync.dma_start(out=X1[:, :], in_=x[b, 1, r0 : r0 + 128, :])
                X1s[t] = X1
            else:
                # x1 folded in during the DMA itself (SWDGE accumulate).
                # Triggered at load-emission time: the trigger's semaphore
                # wait on the x0 load makes it fire as soon as x0 lands,
                # maximizing slack before the DVE needs lum.
                nc.gpsimd.dma_start(
                    out=lum[:, PADL : PADL + W],
                    in_=x[b, 1, r0 : r0 + 128, :],
                    accum_op=mybir.AluOpType.add,
                )

        def st1a_add(t):
            if t < RAMP_T:
                lum = lum_ring[t % NRING]
                nc.vector.tensor_add(
                    lum[:, PADL : PADL + W],
                    lum[:, PADL : PADL + W],
                    X1s.pop(t)[:, :],
                )

        def st1b_add(t):
            # x2 add on DVE: keeps the whole pointwise pool on one engine,
            # avoiding the DVE<->gpsimd SBUF-lockout lockstep.
            lum = lum_ring[t % NRING]
            nc.vector.tensor_add(
                lum[:, PADL : PADL + W],
                lum[:, PADL : PADL + W],
                X2s.pop(t)[:, :],
            )

        def st2_sq(t):
            lum = lum_ring[t % NRING]
            sq = sq_ring[t % NSRING]
            nc.scalar.activation(
                sq[:, PADL : PADL + W],
                lum[:, PADL : PADL + W],
                mybir.ActivationFunctionType.Square,
            )

        def st3_scan(t):
            lum = lum_ring[t % NRING]
            sq = sq_ring[t % NSRING]
            h1 = h1_ring[t % NHRING]
            h2 = h2_ring[t % NHRING]
            # h[:, j] = sum src[j-6..j]; matmul reads h[:, n+3] = centered sum
            for h, src in ((h1, lum), (h2, sq)):
                nc.vector.tensor_tensor_scan(
                    out=h[:, 0:SCAN_N],
                    data0=src[:, PADL : PADL + SCAN_N],
                    data1=src[:, 0:SCAN_N],
                    initial=0.0,
                    op0=mybir.AluOpType.add,
                    op1=mybir.AluOpType.subtract,
                )

        def _banded(S, hring, t, extra=None):
            """S[:, half] = Wmain^T h(t) + Wlo^T h(t-1) + Whi^T h(t+1) [+ extra]."""
            ti = t % 8
            for hf in range(2):
                lo, hi = 3 + 512 * hf, 3 + 512 * hf + 512
                parts = [(0, hring[t % NHRING])]
                if ti > 0:
                    parts.append((1, hring[(t - 1) % NHRING]))
                if ti < 7:
                    parts.append((2, hring[(t + 1) % NHRING]))
                n = len(parts) + (1 if extra is not None else 0)
                for j, (wsel, hsrc) in enumerate(parts):
                    nc.tensor.matmul(
                        S[0:128, 512 * hf : 512 * hf + 512],
                        WB[0:128, 128 * wsel : 128 * wsel + 128],
                        hsrc[0:128, lo:hi],
                        start=(j == 0),
                        stop=(j == n - 1),
                    )
                if extra is not None:
                    nc.tensor.matmul(
                        S[0:128, 512 * hf : 512 * hf + 512],
                        WB[0:128, 384:512],
                        extra[0:128, 512 * hf : 512 * hf + 512],
                        start=False,
                        stop=True,
                    )

        def st4_mm1(t):
            S1 = p1pool.tile([128, W], f32, tag="S1", name=f"S1_{t}")
            _banded(S1, h1_ring, t)
            S1s[t] = S1

        def st5_m2(t):
            m2 = mpool.tile([128, W], f32r, tag="m2", name=f"m2_{t}")
            nc.scalar.activation(
                m2[:, :],
                S1s.pop(t)[:, :],
                mybir.ActivationFunctionType.Square,
                scale=inv147,
            )
            M2s[t] = m2

        def st6_mm2(t):
            S2 = p2pool.tile([128, W], f32, tag="S2", name=f"S2_{t}")
            _banded(S2, h2_ring, t, extra=M2s.pop(t))
            S2s[t] = S2

        def st7_v(t):
            V = vpool.tile([128, W], f32, tag="V", name=f"V_{t}")
            nc.scalar.activation(
                V[:, :],
                S2s.pop(t)[:, :],
                mybir.ActivationFunctionType.Copy,
                scale=inv441,
            )
            Vs[t] = V

        def st8_out(t):
            # ACT-ring HWDGE right after ACT's own V copy: the trigger's
            # data dependency is already satisfied in-order, so it never
            # stalls the scalar queue (unlike a cross-engine wait would).
            b, r0 = img_row(t)
            nc.scalar.dma_start(
                out=y[b, 0, r0 : r0 + 128, :], in_=Vs.pop(t)[0:128, :]
            )

        # Software-pipelined emission with ~2 iterations of slack per stage.
        # Stage s of tile i-s runs in iteration i. On DVE the early-stage
        # x1-add is emitted before the older tile's scans (no head-blocking).
        for i in range(NTILE + 9):
            if i < NTILE:
                st0_load(i)
            if 1 <= i < NTILE + 1:
                st1a_add(i - 1)
            if 2 <= i < NTILE + 2:
                st1b_add(i - 2)
            if 3 <= i < NTILE + 3:
                st2_sq(i - 3)
            if 4 <= i < NTILE + 4:
                st3_scan(i - 4)
            if 6 <= i < NTILE + 6:
                st4_mm1(i - 6)
            if 7 <= i < NTILE + 7:
                st5_m2(i - 7)
            if 8 <= i < NTILE + 8:
                st6_mm2(i - 8)
            if 9 <= i < NTILE + 9:
                st7_v(i - 9)
                st8_out(i - 9)

    if finalize:
        nc.finalize()
    return nc


def kernel(x, kernel_size):
    assert int(kernel_size) == K7
    x = np.ascontiguousarray(np.asarray(x, dtype=np.float32))
    B = x.shape[0]
    assert x.shape == (B, C, H, W) and B == PER_CORE_B * N_CORES

    from concourse.bass_utils import run_bass_kernel_spmd

    nc = build_nc()
    wb = band_weights()
    in_maps = [
        {"x": x[i * PER_CORE_B : (i + 1) * PER_CORE_B], "wb": wb}
        for i in range(N_CORES)
    ]
    res = run_bass_kernel_spmd(nc, in_maps, list(range(N_CORES)))
    y = np.concatenate([res.results[i]["y"] for i in range(N_CORES)], axis=0)
    return y
